# revision 6
# baseline (speedup 1.0000x reference)
"""Trainium2 Bass kernel for nn_Concat_84653805404632.

Reference computation: x is [70, 128, 512] f32; rows 0..19 are supports
(ns_all = n_class*n_support = 20), rows 20..69 are queries (nq_all = 50).
Output [1000, 128, 1024] where out[q*20+s] = concat(sup[s], qry[q], axis=-1).

Pure data movement (memory regime). Sharding: the (query, support) pair grid
[50 x 20] is split as (2 query-halves) x (4 support-fifths) -> 8 cores, each
producing exactly 125 output rows (64 MB) with an identical SPMD access
pattern.

Per core: the 5 support tiles are DMA-loaded directly into the sup columns of
two interleaved "image" buffers in SBUF; the VectorEngine broadcasts each
query tile into the qry columns (SBUF engine ports are separate from the DMA
AXI ports, so this is free); each query then leaves as ONE 2.62 MB write DMA
whose descriptors are full 4 KB rows. 4 KB descriptors matter: SDMA engine 15
has ~15 ns/packet extra fixed cost and the in-order descriptor generator
stalls on its ring, so the whole core runs at engine-15's packet rate —
bigger packets raise that ceiling from ~325 GB/s to ~390 GB/s.
"""

import os
import sys

import numpy as np

for _p in ("/opt/trn_rl_repo", "/root/.axon_site/_ro/trn_rl_repo"):
    if os.path.isdir(_p) and _p not in sys.path:
        sys.path.insert(0, _p)

import concourse.bass as bass
import concourse.mybir as mybir
from concourse.bass_utils import run_bass_kernel_spmd

NS_ALL = 20  # n_class * n_support
NQ_ALL = 50  # n_class * n_query
D = 128
F = 512
QH = 25  # queries per core  (NQ_ALL / 2)
SF = 5  # supports per core (NS_ALL / 4)
QCH = 5  # query tiles per load chunk
PKEEP = 127  # partitions written from SBUF; column 127 spilled from DRAM
N_CORES = 8

_NC_CACHE = None


def _build_nc():
    nc = bass.Bass()
    sup = nc.declare_dram_parameter("sup", [SF, D, F], mybir.dt.float32, isOutput=False)
    qry = nc.declare_dram_parameter("qry", [QH, D, F], mybir.dt.float32, isOutput=False)
    out = nc.declare_dram_parameter(
        "out", [QH * SF, D, 2 * F], mybir.dt.float32, isOutput=True
    )

    with (
        nc.sbuf_tensor([D, QH * F], mybir.dt.float32) as qry_t,
        nc.sbuf_tensor([D, SF * 2 * F], mybir.dt.float32) as img0,
        nc.sbuf_tensor([D, SF * 2 * F], mybir.dt.float32) as img1,
        nc.semaphore("img_sup_sem0") as img_sup_sem0,
        nc.semaphore("qry_sem0") as qry_sem0,
        nc.semaphore("qry_sem1") as qry_sem1,
        nc.semaphore("qry_sem2") as qry_sem2,
        nc.semaphore("qry_sem3") as qry_sem3,
        nc.semaphore("qry_sem4") as qry_sem4,
        nc.semaphore("spill_sem") as spill_sem,
        nc.semaphore("dve_sem") as dve_sem,
        nc.semaphore("out_sem0") as out_sem0,
        nc.semaphore("out_sem1") as out_sem1,
        nc.Block() as block,
    ):
        imgs = [img0, img1]
        qry_sems = [qry_sem0, qry_sem1, qry_sem2, qry_sem3, qry_sem4]
        out_sems = [out_sem0, out_sem1]

        def img_view(b):
            return imgs[b][:].rearrange("p (s f2) -> p s f2", f2=2 * F)

        @block.sync
        def _(sync):
            # sup tiles into img0's sup columns; DVE mirrors them to img1.
            sync.dma_start(
                img_view(0)[:, :, 0:F], sup[:].transpose([1, 0, 2])
            ).then_inc(img_sup_sem0, 16)
            sync.dma_start(
                qry_t[:, 0 : QCH * F],
                qry[0:QCH].transpose([1, 0, 2]),
            ).then_inc(qry_sems[0], 16)
            # Column-127 spill: write the excluded column for ALL output rows
            # straight from DRAM inputs with broadcast (stride-0) sources.
            # HWDGE ring only — on gpsimd (SWDGE) the DVE's 2-port copies
            # lock it out of the descriptor rings and the spill crawls.
            sup_spill_src = (
                sup[0:SF, 127, 0:F].unsqueeze(0).broadcast_to([QH, SF, F])
            )
            sync.dma_start(out[:, 127, 0:F], sup_spill_src).then_inc(spill_sem, 16)
            qry_spill_src = (
                qry[0:QH, 127, 0:F].unsqueeze(1).broadcast_to([QH, SF, F])
            )
            sync.dma_start(out[:, 127, F : 2 * F], qry_spill_src).then_inc(
                spill_sem, 16
            )
            for c in range(1, QH // QCH):
                sync.dma_start(
                    qry_t[:, QCH * F * c : QCH * F * (c + 1)],
                    qry[QCH * c : QCH * (c + 1)].transpose([1, 0, 2]),
                ).then_inc(qry_sems[c], 16)

        @block.vector
        def _(vector):
            # op 1: mirror sup columns img0 -> img1
            vector.wait_ge(img_sup_sem0, 16)
            vector.tensor_copy(
                img_view(1)[:, :, 0:F], img_view(0)[:, :, 0:F]
            ).then_inc(dve_sem, 1)
            # ops 2..26: query broadcast into image q%2
            for q in range(QH):
                vector.wait_ge(qry_sems[q // QCH], 16)
                if q >= 2:
                    vector.wait_ge(out_sems[q % 2], 16 * (q // 2))
                dst = img_view(q % 2)[:, :, F : 2 * F]
                src = (
                    qry_t[:, F * q : F * (q + 1)]
                    .unsqueeze(1)
                    .broadcast_to([D, SF, F])
                )
                vector.tensor_copy(dst, src).then_inc(dve_sem, 1)

        @block.scalar
        def _(scalar):
            for q in range(QH):
                if q == 0:
                    scalar.wait_ge(img_sup_sem0, 16)
                scalar.wait_ge(dve_sem, q + 2)
                dst = out[SF * q : SF * (q + 1), :, :].transpose([1, 0, 2])[0:PKEEP]
                scalar.dma_start(dst, imgs[q % 2][0:PKEEP, :]).then_inc(
                    out_sems[q % 2], 16
                )
            scalar.wait_ge(out_sem0, 16 * ((QH + 1) // 2))
            scalar.wait_ge(out_sem1, 16 * (QH // 2))
            scalar.wait_ge(spill_sem, 32)

    return nc


def _get_nc():
    global _NC_CACHE
    if _NC_CACHE is None:
        _NC_CACHE = _build_nc()
    return _NC_CACHE


def kernel(**inputs) -> np.ndarray:
    x = np.ascontiguousarray(np.asarray(inputs["x"], dtype=np.float32))
    assert x.shape == (NS_ALL + NQ_ALL, D, F), x.shape

    sup_all = x[:NS_ALL]
    qry_all = x[NS_ALL:]

    in_maps = []
    for k in range(N_CORES):
        h, f = divmod(k, 4)
        in_maps.append(
            {
                "sup": np.ascontiguousarray(sup_all[SF * f : SF * (f + 1)]),
                "qry": np.ascontiguousarray(qry_all[QH * h : QH * (h + 1)]),
            }
        )

    nc = _get_nc()
    res = run_bass_kernel_spmd(nc, in_maps, core_ids=list(range(N_CORES)))

    full = np.empty((NQ_ALL, NS_ALL, D, 2 * F), dtype=np.float32)
    for k in range(N_CORES):
        h, f = divmod(k, 4)
        out_k = np.asarray(res.results[k]["out"]).reshape(QH, SF, D, 2 * F)
        full[QH * h : QH * (h + 1), SF * f : SF * (f + 1)] = out_k
    return full.reshape(NQ_ALL * NS_ALL, D, 2 * F)


# revision 9
# speedup vs baseline: 9.0615x; 9.0615x over previous
"""Trainium2 Bass kernel for nn_Concat_84653805404632.

Reference computation: x is [70, 128, 512] f32; rows 0..19 are supports
(ns_all = n_class*n_support = 20), rows 20..69 are queries (nq_all = 50).
Output [1000, 128, 1024] where out[q*20+s] = concat(sup[s], qry[q], axis=-1).

Pure data movement (memory regime). Sharding: the (query, support) pair grid
[50 x 20] is split as (2 query-halves) x (4 support-fifths) -> 8 cores, each
producing exactly 125 output rows (64 MB) with an identical SPMD access
pattern.

Per core: the 5 support tiles are DMA-loaded directly into the sup columns of
two interleaved "image" buffers in SBUF; the VectorEngine broadcasts each
query tile into the qry columns (SBUF engine ports are separate from the DMA
AXI ports, so this is free); each query then leaves as ONE 2.62 MB write DMA
whose descriptors are full 4 KB rows. 4 KB descriptors matter: SDMA engine 15
has ~15 ns/packet extra fixed cost and the in-order descriptor generator
stalls on its ring, so the whole core runs at engine-15's packet rate —
bigger packets raise that ceiling from ~325 GB/s to ~390 GB/s.
"""

import contextlib
import os
import sys

import numpy as np

for _p in ("/opt/trn_rl_repo", "/root/.axon_site/_ro/trn_rl_repo"):
    if os.path.isdir(_p) and _p not in sys.path:
        sys.path.insert(0, _p)

import concourse.bass as bass
import concourse.mybir as mybir
from concourse.bass_utils import run_bass_kernel_spmd

NS_ALL = 20  # n_class * n_support
NQ_ALL = 50  # n_class * n_query
D = 128
F = 512
QH = 25  # queries per core  (NQ_ALL / 2)
SF = 5  # supports per core (NS_ALL / 4)
QCH = 5  # query tiles per load chunk
SPL0 = 124  # first spilled column
NSPL = 4  # spilled columns 124..127
QHALF = 13  # q-half split for spare-partition sets
N_CORES = 8

_NC_CACHE = None


def _build_nc():
    nc = bass.Bass()
    sup = nc.declare_dram_parameter("sup", [SF, D, F], mybir.dt.float32, isOutput=False)
    qry = nc.declare_dram_parameter("qry", [QH, D, F], mybir.dt.float32, isOutput=False)
    out = nc.declare_dram_parameter(
        "out", [QH * SF, D, 2 * F], mybir.dt.float32, isOutput=True
    )

    with contextlib.ExitStack() as _stk:
        qry_t = _stk.enter_context(nc.sbuf_tensor([D, QH * F], mybir.dt.float32))
        img0 = _stk.enter_context(nc.sbuf_tensor([D, SF * 2 * F], mybir.dt.float32))
        img1 = _stk.enter_context(nc.sbuf_tensor([D, SF * 2 * F], mybir.dt.float32))
        spl_q0 = _stk.enter_context(nc.sbuf_tensor([D, QHALF * F], mybir.dt.float32))
        spl_q1 = _stk.enter_context(nc.sbuf_tensor([D, QHALF * F], mybir.dt.float32))
        spl_s0 = _stk.enter_context(nc.sbuf_tensor([D, SF * F], mybir.dt.float32))
        spl_s1 = _stk.enter_context(nc.sbuf_tensor([D, SF * F], mybir.dt.float32))
        img_sup_sem0 = _stk.enter_context(nc.semaphore("img_sup_sem0"))
        qry_sems = [_stk.enter_context(nc.semaphore(f"qry_sem{i}")) for i in range(5)]
        splld = [_stk.enter_context(nc.semaphore(f"splld_{i}")) for i in range(4)]
        spill_wr = _stk.enter_context(nc.semaphore("spill_wr"))
        dve_sem = _stk.enter_context(nc.semaphore("dve_sem"))
        outa = [_stk.enter_context(nc.semaphore(f"out_a{i}")) for i in range(2)]
        outb = [_stk.enter_context(nc.semaphore(f"out_b{i}")) for i in range(2)]
        block = _stk.enter_context(nc.Block())

        imgs = [img0, img1]
        spl_qs = [spl_q0, spl_q1]
        spl_ss = [spl_s0, spl_s1]

        def img_view(b):
            return imgs[b][:].rearrange("p (s f2) -> p s f2", f2=2 * F)

        def spares(h):
            return slice(64 * h, 64 * h + 32, 8)

        @block.scalar
        def _(scalar):
            scalar.dma_start(
                img_view(0)[:, :, 0:F], sup[:].transpose([1, 0, 2])
            ).then_inc(img_sup_sem0, 16)
            scalar.dma_start(
                qry_t[:, 0 : QCH * F], qry[0:QCH].transpose([1, 0, 2])
            ).then_inc(qry_sems[0], 16)
            nqh = [QHALF, QH - QHALF]
            for h in range(2):
                scalar.dma_start(
                    spl_qs[h][spares(h), 0 : nqh[h] * F],
                    qry[
                        QHALF * h : QHALF * h + nqh[h], SPL0 : SPL0 + NSPL, :
                    ].transpose([1, 0, 2]),
                ).then_inc(splld[h], 16)
                scalar.dma_start(
                    spl_ss[h][spares(h), :],
                    sup[:, SPL0 : SPL0 + NSPL, :].transpose([1, 0, 2]),
                ).then_inc(splld[2 + h], 16)
            for q in range(QH):
                if q == 0:
                    scalar.wait_ge(img_sup_sem0, 16)
                scalar.wait_ge(dve_sem, q + 2)
                dstT = out[SF * q : SF * (q + 1), :, :].transpose([1, 0, 2])
                scalar.dma_start(dstT[0:64], imgs[q % 2][0:64, :]).then_inc(
                    outa[q % 2], 16
                )
            scalar.wait_ge(outa[0], 16 * ((QH + 1) // 2))
            scalar.wait_ge(outa[1], 16 * (QH // 2))

        @block.sync
        def _(sync):
            for c in range(1, QH // QCH):
                sync.dma_start(
                    qry_t[:, QCH * F * c : QCH * F * (c + 1)],
                    qry[QCH * c : QCH * (c + 1)].transpose([1, 0, 2]),
                ).then_inc(qry_sems[c], 16)
            for s in splld:
                sync.wait_ge(s, 16)
            for q in range(QH):
                h = 0 if q < QHALF else 1
                dst_s = out[
                    SF * q : SF * (q + 1), SPL0 : SPL0 + NSPL, 0:F
                ].transpose([1, 0, 2])
                src_s = spl_ss[h][spares(h), :].rearrange("p (s f) -> p s f", f=F)
                sync.dma_start(dst_s, src_s).then_inc(spill_wr, 16)
                dst_q = out[
                    SF * q : SF * (q + 1), SPL0 : SPL0 + NSPL, F : 2 * F
                ].transpose([1, 0, 2])
                qq = q - QHALF * h
                src_q = (
                    spl_qs[h][spares(h), F * qq : F * (qq + 1)]
                    .unsqueeze(1)
                    .broadcast_to([NSPL, SF, F])
                )
                sync.dma_start(dst_q, src_q).then_inc(spill_wr, 16)
            for q in range(QH):
                sync.wait_ge(dve_sem, q + 2)
                dstT = out[SF * q : SF * (q + 1), :, :].transpose([1, 0, 2])
                sync.dma_start(dstT[64:SPL0], imgs[q % 2][64:SPL0, :]).then_inc(
                    outb[q % 2], 16
                )
            sync.wait_ge(outb[0], 16 * ((QH + 1) // 2))
            sync.wait_ge(outb[1], 16 * (QH // 2))
            sync.wait_ge(spill_wr, 16 * 2 * QH)

        @block.vector
        def _(vector):
            vector.wait_ge(img_sup_sem0, 16)
            vector.tensor_copy(
                img_view(1)[:, :, 0:F], img_view(0)[:, :, 0:F]
            ).then_inc(dve_sem, 1)
            for q in range(QH):
                vector.wait_ge(qry_sems[q // QCH], 16)
                if q >= 2:
                    vector.wait_ge(outa[q % 2], 16 * (q // 2))
                    vector.wait_ge(outb[q % 2], 16 * (q // 2))
                dst = img_view(q % 2)[:, :, F : 2 * F]
                src = (
                    qry_t[:, F * q : F * (q + 1)]
                    .unsqueeze(1)
                    .broadcast_to([D, SF, F])
                )
                vector.tensor_copy(dst, src).then_inc(dve_sem, 1)

    return nc


def _get_nc():
    global _NC_CACHE
    if _NC_CACHE is None:
        _NC_CACHE = _build_nc()
    return _NC_CACHE


def kernel(**inputs) -> np.ndarray:
    x = np.ascontiguousarray(np.asarray(inputs["x"], dtype=np.float32))
    assert x.shape == (NS_ALL + NQ_ALL, D, F), x.shape

    sup_all = x[:NS_ALL]
    qry_all = x[NS_ALL:]

    in_maps = []
    for k in range(N_CORES):
        h, f = divmod(k, 4)
        in_maps.append(
            {
                "sup": np.ascontiguousarray(sup_all[SF * f : SF * (f + 1)]),
                "qry": np.ascontiguousarray(qry_all[QH * h : QH * (h + 1)]),
            }
        )

    nc = _get_nc()
    res = run_bass_kernel_spmd(nc, in_maps, core_ids=list(range(N_CORES)))

    full = np.empty((NQ_ALL, NS_ALL, D, 2 * F), dtype=np.float32)
    for k in range(N_CORES):
        h, f = divmod(k, 4)
        out_k = np.asarray(res.results[k]["out"]).reshape(QH, SF, D, 2 * F)
        full[QH * h : QH * (h + 1), SF * f : SF * (f + 1)] = out_k
    return full.reshape(NQ_ALL * NS_ALL, D, 2 * F)


# revision 10
# speedup vs baseline: 12.8249x; 1.4153x over previous
"""Trainium2 Bass kernel for nn_Concat_84653805404632.

Reference computation: x is [70, 128, 512] f32; rows 0..19 are supports
(ns_all = n_class*n_support = 20), rows 20..69 are queries (nq_all = 50).
Output [1000, 128, 1024] where out[q*20+s] = concat(sup[s], qry[q], axis=-1).

Pure data movement (memory regime). Sharding: the (query, support) pair grid
[50 x 20] is split as (2 query-halves) x (4 support-fifths) -> 8 cores, each
producing exactly 125 output rows (64 MB) with an identical SPMD access
pattern.

Per core: the 5 support tiles are DMA-loaded directly into the sup columns of
two interleaved "image" buffers in SBUF; the VectorEngine broadcasts each
query tile into the qry columns (SBUF engine ports are separate from the DMA
AXI ports, so this is free); each query then leaves as ONE 2.62 MB write DMA
whose descriptors are full 4 KB rows. 4 KB descriptors matter: SDMA engine 15
has ~15 ns/packet extra fixed cost and the in-order descriptor generator
stalls on its ring, so the whole core runs at engine-15's packet rate —
bigger packets raise that ceiling from ~325 GB/s to ~390 GB/s.
"""

import os
import sys

import numpy as np

for _p in ("/opt/trn_rl_repo", "/root/.axon_site/_ro/trn_rl_repo"):
    if os.path.isdir(_p) and _p not in sys.path:
        sys.path.insert(0, _p)

import concourse.bass as bass
import concourse.mybir as mybir
from concourse.bass_utils import run_bass_kernel_spmd

NS_ALL = 20  # n_class * n_support
NQ_ALL = 50  # n_class * n_query
D = 128
F = 512
QH = 25  # queries per core  (NQ_ALL / 2)
SF = 5  # supports per core (NS_ALL / 4)
QCH = 5  # query tiles per load chunk
N_CORES = 8

_NC_CACHE = None


def _build_nc():
    nc = bass.Bass()
    sup = nc.declare_dram_parameter("sup", [SF, D, F], mybir.dt.float32, isOutput=False)
    qry = nc.declare_dram_parameter("qry", [QH, D, F], mybir.dt.float32, isOutput=False)
    out = nc.declare_dram_parameter(
        "out", [QH * SF, D, 2 * F], mybir.dt.float32, isOutput=True
    )

    with (
        nc.sbuf_tensor([D, QH * F], mybir.dt.float32) as qry_t,
        nc.sbuf_tensor([D, SF * 2 * F], mybir.dt.float32) as img0,
        nc.sbuf_tensor([D, SF * 2 * F], mybir.dt.float32) as img1,
        nc.semaphore("img_sup_sem0") as img_sup_sem0,
        nc.semaphore("img_sup_sem1") as img_sup_sem1,
        nc.semaphore("qry_sem0") as qry_sem0,
        nc.semaphore("qry_sem1") as qry_sem1,
        nc.semaphore("qry_sem2") as qry_sem2,
        nc.semaphore("qry_sem3") as qry_sem3,
        nc.semaphore("qry_sem4") as qry_sem4,
        nc.semaphore("dve_sem") as dve_sem,
        nc.semaphore("out_sem0") as out_sem0,
        nc.semaphore("out_sem1") as out_sem1,
        nc.Block() as block,
    ):
        imgs = [img0, img1]
        sup_sems = [img_sup_sem0, img_sup_sem1]
        qry_sems = [qry_sem0, qry_sem1, qry_sem2, qry_sem3, qry_sem4]
        out_sems = [out_sem0, out_sem1]

        def img_view(b):
            # [p, s, f2] view of the 5-row interleaved image (f2 = 1024)
            return imgs[b][:].rearrange("p (s f2) -> p s f2", f2=2 * F)

        @block.sync
        def _(sync):
            # Support tiles straight into the sup columns of both images,
            # then the query tiles in chunks (per-chunk semaphores: DMA
            # completions are unordered).
            sync.dma_start(
                img_view(0)[:, :, 0:F], sup[:].transpose([1, 0, 2])
            ).then_inc(sup_sems[0], 16)
            sync.dma_start(
                qry_t[:, 0 : QCH * F], qry[0:QCH].transpose([1, 0, 2])
            ).then_inc(qry_sems[0], 16)
            sync.dma_start(
                img_view(1)[:, :, 0:F], sup[:].transpose([1, 0, 2])
            ).then_inc(sup_sems[1], 16)
            for c in range(1, QH // QCH):
                sync.dma_start(
                    qry_t[:, QCH * F * c : QCH * F * (c + 1)],
                    qry[QCH * c : QCH * (c + 1)].transpose([1, 0, 2]),
                ).then_inc(qry_sems[c], 16)

        @block.vector
        def _(vector):
            for q in range(QH):
                vector.wait_ge(qry_sems[q // QCH], 16)
                if q >= 2:
                    # img[q%2] free once all issued writes on it are done.
                    vector.wait_ge(out_sems[q % 2], 16 * (q // 2))
                dst = img_view(q % 2)[:, :, F : 2 * F]
                src = (
                    qry_t[:, F * q : F * (q + 1)]
                    .unsqueeze(1)
                    .broadcast_to([D, SF, F])
                )
                vector.tensor_copy(dst, src).then_inc(dve_sem, 1)

        @block.scalar
        def _(scalar):
            for q in range(QH):
                if q == 0:
                    scalar.wait_ge(img_sup_sem0, 16)
                elif q == 1:
                    scalar.wait_ge(img_sup_sem1, 16)
                scalar.wait_ge(dve_sem, q + 1)
                dst = out[SF * q : SF * (q + 1), :, :].transpose([1, 0, 2])
                scalar.dma_start(dst, imgs[q % 2][:]).then_inc(out_sems[q % 2], 16)
            scalar.wait_ge(out_sem0, 16 * ((QH + 1) // 2))
            scalar.wait_ge(out_sem1, 16 * (QH // 2))

    return nc


def _get_nc():
    global _NC_CACHE
    if _NC_CACHE is None:
        _NC_CACHE = _build_nc()
    return _NC_CACHE


def kernel(**inputs) -> np.ndarray:
    x = np.ascontiguousarray(np.asarray(inputs["x"], dtype=np.float32))
    assert x.shape == (NS_ALL + NQ_ALL, D, F), x.shape

    sup_all = x[:NS_ALL]
    qry_all = x[NS_ALL:]

    in_maps = []
    for k in range(N_CORES):
        h, f = divmod(k, 4)
        in_maps.append(
            {
                "sup": np.ascontiguousarray(sup_all[SF * f : SF * (f + 1)]),
                "qry": np.ascontiguousarray(qry_all[QH * h : QH * (h + 1)]),
            }
        )

    nc = _get_nc()
    res = run_bass_kernel_spmd(nc, in_maps, core_ids=list(range(N_CORES)))

    full = np.empty((NQ_ALL, NS_ALL, D, 2 * F), dtype=np.float32)
    for k in range(N_CORES):
        h, f = divmod(k, 4)
        out_k = np.asarray(res.results[k]["out"]).reshape(QH, SF, D, 2 * F)
        full[QH * h : QH * (h + 1), SF * f : SF * (f + 1)] = out_k
    return full.reshape(NQ_ALL * NS_ALL, D, 2 * F)


# revision 11
# speedup vs baseline: 12.8601x; 1.0027x over previous
"""Trainium2 Bass kernel for nn_Concat_84653805404632.

Reference computation: x is [70, 128, 512] f32; rows 0..19 are supports
(ns_all = n_class*n_support = 20), rows 20..69 are queries (nq_all = 50).
Output [1000, 128, 1024] where out[q*20+s] = concat(sup[s], qry[q], axis=-1).

Pure data movement (memory regime). Sharding: the (query, support) pair grid
[50 x 20] is split as (2 query-halves) x (4 support-fifths) -> 8 cores, each
producing exactly 125 output rows (64 MB) with an identical SPMD access
pattern.

Per core: the 5 support tiles are DMA-loaded directly into the sup columns of
two interleaved "image" buffers in SBUF; the VectorEngine broadcasts each
query tile into the qry columns (SBUF engine ports are separate from the DMA
AXI ports, so this overlaps the writes for free); each query then leaves as
ONE 2.62 MB write DMA whose descriptors are full 4 KB rows — the largest
descriptor this output layout allows, which keeps all 16 SDMA engines at
their ~25 B/ns per-descriptor rate (~400-420 GB/s per core, the 16-engine
descriptor-processing ceiling). Writes double-buffer against the DVE copies;
the load order (sup image 0, first query chunk, sup image 1, remaining
chunks) lets the first write start as early as possible.

Measured on 8 trn2 cores: 195 us NEFF exec, rel err 0 (the 64 MB/core write
floor at ~403 GB/s is ~185 us plus ~11 us NEFF startup/teardown).
"""

import os
import sys

import numpy as np

for _p in ("/opt/trn_rl_repo", "/root/.axon_site/_ro/trn_rl_repo"):
    if os.path.isdir(_p) and _p not in sys.path:
        sys.path.insert(0, _p)

import concourse.bass as bass
import concourse.mybir as mybir
from concourse.bass_utils import run_bass_kernel_spmd

NS_ALL = 20  # n_class * n_support
NQ_ALL = 50  # n_class * n_query
D = 128
F = 512
QH = 25  # queries per core  (NQ_ALL / 2)
SF = 5  # supports per core (NS_ALL / 4)
QCH = 5  # query tiles per load chunk
N_CORES = 8

_NC_CACHE = None


def _build_nc():
    nc = bass.Bass()
    sup = nc.declare_dram_parameter("sup", [SF, D, F], mybir.dt.float32, isOutput=False)
    qry = nc.declare_dram_parameter("qry", [QH, D, F], mybir.dt.float32, isOutput=False)
    out = nc.declare_dram_parameter(
        "out", [QH * SF, D, 2 * F], mybir.dt.float32, isOutput=True
    )

    with (
        nc.sbuf_tensor([D, QH * F], mybir.dt.float32) as qry_t,
        nc.sbuf_tensor([D, SF * 2 * F], mybir.dt.float32) as img0,
        nc.sbuf_tensor([D, SF * 2 * F], mybir.dt.float32) as img1,
        nc.semaphore("img_sup_sem0") as img_sup_sem0,
        nc.semaphore("img_sup_sem1") as img_sup_sem1,
        nc.semaphore("qry_sem0") as qry_sem0,
        nc.semaphore("qry_sem1") as qry_sem1,
        nc.semaphore("qry_sem2") as qry_sem2,
        nc.semaphore("qry_sem3") as qry_sem3,
        nc.semaphore("qry_sem4") as qry_sem4,
        nc.semaphore("dve_sem") as dve_sem,
        nc.semaphore("out_sem0") as out_sem0,
        nc.semaphore("out_sem1") as out_sem1,
        nc.Block() as block,
    ):
        imgs = [img0, img1]
        sup_sems = [img_sup_sem0, img_sup_sem1]
        qry_sems = [qry_sem0, qry_sem1, qry_sem2, qry_sem3, qry_sem4]
        out_sems = [out_sem0, out_sem1]

        def img_view(b):
            # [p, s, f2] view of the 5-row interleaved image (f2 = 1024)
            return imgs[b][:].rearrange("p (s f2) -> p s f2", f2=2 * F)

        @block.sync
        def _(sync):
            # Support tiles straight into the sup columns of both images,
            # then the query tiles in chunks (per-chunk semaphores: DMA
            # completions are unordered).
            sync.dma_start(
                img_view(0)[:, :, 0:F], sup[:].transpose([1, 0, 2])
            ).then_inc(sup_sems[0], 16)
            sync.dma_start(
                qry_t[:, 0 : QCH * F], qry[0:QCH].transpose([1, 0, 2])
            ).then_inc(qry_sems[0], 16)
            sync.dma_start(
                img_view(1)[:, :, 0:F], sup[:].transpose([1, 0, 2])
            ).then_inc(sup_sems[1], 16)
            for c in range(1, QH // QCH):
                sync.dma_start(
                    qry_t[:, QCH * F * c : QCH * F * (c + 1)],
                    qry[QCH * c : QCH * (c + 1)].transpose([1, 0, 2]),
                ).then_inc(qry_sems[c], 16)

        @block.vector
        def _(vector):
            for q in range(QH):
                vector.wait_ge(qry_sems[q // QCH], 16)
                if q >= 2:
                    # img[q%2] free once all issued writes on it are done.
                    vector.wait_ge(out_sems[q % 2], 16 * (q // 2))
                dst = img_view(q % 2)[:, :, F : 2 * F]
                src = (
                    qry_t[:, F * q : F * (q + 1)]
                    .unsqueeze(1)
                    .broadcast_to([D, SF, F])
                )
                vector.tensor_copy(dst, src).then_inc(dve_sem, 1)

        @block.scalar
        def _(scalar):
            for q in range(QH):
                if q == 0:
                    scalar.wait_ge(img_sup_sem0, 16)
                elif q == 1:
                    scalar.wait_ge(img_sup_sem1, 16)
                scalar.wait_ge(dve_sem, q + 1)
                dst = out[SF * q : SF * (q + 1), :, :].transpose([1, 0, 2])
                scalar.dma_start(dst, imgs[q % 2][:]).then_inc(out_sems[q % 2], 16)
            scalar.wait_ge(out_sem0, 16 * ((QH + 1) // 2))
            scalar.wait_ge(out_sem1, 16 * (QH // 2))

    return nc


def _get_nc():
    global _NC_CACHE
    if _NC_CACHE is None:
        _NC_CACHE = _build_nc()
    return _NC_CACHE


def kernel(**inputs) -> np.ndarray:
    x = np.ascontiguousarray(np.asarray(inputs["x"], dtype=np.float32))
    assert x.shape == (NS_ALL + NQ_ALL, D, F), x.shape

    sup_all = x[:NS_ALL]
    qry_all = x[NS_ALL:]

    in_maps = []
    for k in range(N_CORES):
        h, f = divmod(k, 4)
        in_maps.append(
            {
                "sup": np.ascontiguousarray(sup_all[SF * f : SF * (f + 1)]),
                "qry": np.ascontiguousarray(qry_all[QH * h : QH * (h + 1)]),
            }
        )

    nc = _get_nc()
    res = run_bass_kernel_spmd(nc, in_maps, core_ids=list(range(N_CORES)))

    full = np.empty((NQ_ALL, NS_ALL, D, 2 * F), dtype=np.float32)
    for k in range(N_CORES):
        h, f = divmod(k, 4)
        out_k = np.asarray(res.results[k]["out"]).reshape(QH, SF, D, 2 * F)
        full[QH * h : QH * (h + 1), SF * f : SF * (f + 1)] = out_k
    return full.reshape(NQ_ALL * NS_ALL, D, 2 * F)
